# revision 1
# baseline (speedup 1.0000x reference)
"""Distributed 2-layer GCN (GCLEncoder) on 8 Trainium2 NeuronCores — Bass/Tile.

kernel(**inputs) takes the FULL inputs (x [100000,128] f32, W1 [128,64],
b1 [64], W2 [64,32], b2 [32], edge_index [2,1600000] i32) and returns the
FULL output z [100000, 32] f32.

Destination nodes are sharded contiguously across the 8 cores; per-layer
feature tables G = dinv*(H@W) are node-sharded and exchanged via AllGather.
Within a core, nodes are assigned to 256-destination groups by a
degree-balancing permutation so every group needs the same minimal number
of 128-edge blocks (TB = ceil(mean/128)); the host un-permutes the output
rows.  Per block: indirect-DMA gather of referenced table rows (128
rows/instruction — the only gather primitive on this runtime), a 256-wide
one-hot S on the vector engine, and two M=128 matmuls (lo/hi halves)
accumulating in PSUM.  Self-loops are explicit edges; degrees are
host-computed index metadata; all float math runs on device.
"""


from dataclasses import dataclass

import numpy as np

import concourse.bass as bass
import concourse.tile as tile
import concourse.bacc as bacc
from concourse import bass_utils, mybir
from concourse.masks import make_identity

F32 = mybir.dt.float32
I32 = mybir.dt.int32
P = 128
WG = 256  # dst group size


@dataclass(frozen=True)
class Cfg2:
    n_nodes: int
    din: int
    dh: int
    dout: int
    C: int
    NG: int  # 256-dst groups per core
    TB: int  # 128-slot blocks per group

    @property
    def npc(self):
        return self.n_nodes // self.C

    @property
    def npcp(self):
        return self.NG * WG

    @property
    def NT(self):  # 128-node tiles per core
        return self.NG * 2

    @property
    def NB(self):
        return self.NG * self.TB


def build_schedule(edge_index: np.ndarray, n_nodes: int, C: int):
    src = np.asarray(edge_index[0], dtype=np.int64)
    dst = np.asarray(edge_index[1], dtype=np.int64)
    loops = np.arange(n_nodes, dtype=np.int64)
    src = np.concatenate([src, loops])
    dst = np.concatenate([dst, loops])

    npc = n_nodes // C
    NG = -(-npc // WG)
    npcp = NG * WG

    deg = np.bincount(dst, minlength=n_nodes).astype(np.float32)
    owner = dst // npc

    # balance nodes into groups by degree so the per-group padded block
    # count TB = ceil(max group count / 128) sits at the mean, not the max
    import heapq

    pos_of_node = np.zeros((C, npc), dtype=np.int64)
    node_of_pos = np.full((C, npcp), -1, dtype=np.int64)
    for c in range(C):
        dc = deg[c * npc : (c + 1) * npc].astype(np.int64)
        order = np.argsort(-dc, kind="stable")
        sums = [0.0] * NG
        cnts = [0] * NG
        heap = [(0.0, b) for b in range(NG)]
        heapq.heapify(heap)
        binof = np.zeros(npc, dtype=np.int64)
        for nidx in order:
            while True:
                sm, b = heapq.heappop(heap)
                if cnts[b] < WG:
                    break
            binof[nidx] = b
            cnts[b] += 1
            sums[b] += dc[nidx]
            if cnts[b] < WG:
                heapq.heappush(heap, (sums[b], b))
        # refinement: swap between max and min bins to shave the max
        for _ in range(4000):
            bmax = int(np.argmax(sums))
            bmin = int(np.argmin(sums))
            gap = sums[bmax] - sums[bmin]
            if gap <= 1:
                break
            in_max = np.where(binof == bmax)[0]
            in_min = np.where(binof == bmin)[0]
            du = dc[in_max]
            dv = dc[in_min]
            diff = du[:, None] - dv[None, :]
            good = (diff > 0) & (diff < gap)
            if not good.any():
                break
            tgt = gap / 2.0
            score = np.where(good, np.abs(diff - tgt), np.inf)
            iu, iv = np.unravel_index(np.argmin(score), score.shape)
            u, v = in_max[iu], in_min[iv]
            d_ = int(dc[u] - dc[v])
            binof[u], binof[v] = bmin, bmax
            sums[bmax] -= d_
            sums[bmin] += d_
        # positions: pack each bin's nodes then ghosts
        slot = np.zeros(NG, dtype=np.int64)
        for nidx in range(npc):
            b = binof[nidx]
            p = b * WG + slot[b]
            slot[b] += 1
            pos_of_node[c, nidx] = p
            node_of_pos[c, p] = nidx

    per_core = []
    maxcnt = 0
    for c in range(C):
        m = owner == c
        s_glob = src[m]
        d = dst[m] - c * npc
        pd = pos_of_node[c, d]
        order = np.argsort(pd, kind="stable")
        s_glob, pd = s_glob[order], pd[order]
        t_idx = (s_glob // npc) * npcp + pos_of_node[s_glob // npc, s_glob % npc]
        grp = pd // WG
        counts = np.bincount(grp, minlength=NG)
        maxcnt = max(maxcnt, int(counts.max()))
        per_core.append((t_idx, pd, grp, counts))

    TB = max(1, -(-maxcnt // P))
    cfg = Cfg2(n_nodes=n_nodes, din=128, dh=64, dout=32, C=C, NG=NG, TB=TB)

    data = []
    for c in range(C):
        t_idx, pd, grp, counts = per_core[c]
        NB = NG * TB
        src_idx = np.zeros((P, NB), dtype=np.int32)
        dst_rel = np.full((P, NB), -1.0, dtype=np.float32)
        starts = np.cumsum(counts) - counts
        pos = np.arange(len(pd)) - starts[grp]
        col = grp * TB + pos // P
        row = pos % P
        src_idx[row, col] = t_idx.astype(np.int32)
        dst_rel[row, col] = (pd - grp * WG).astype(np.float32)

        deg_nm = np.ones((P, NG * 2), dtype=np.float32)
        valid = node_of_pos[c] >= 0
        dpad = np.ones(npcp, np.float32)
        dpad[valid] = deg[c * npc + node_of_pos[c][valid]]
        deg_nm[:, :] = dpad.reshape(NG * 2, P).T
        data.append(
            {
                "src_idx": src_idx,
                "dst_rel": dst_rel,
                "deg_nm": deg_nm,
                "pos_of_node": pos_of_node[c],
            }
        )
    return cfg, data


def build_inputs(cfg: Cfg2, x, W1, b1, W2, b2, sched):
    C, npc, npcp = cfg.C, cfg.npc, cfg.npcp
    x = np.asarray(x, dtype=np.float32)
    b1r = np.tile(np.asarray(b1, np.float32)[None, :], (P, 1))
    b2r = np.tile(np.asarray(b2, np.float32)[None, :], (P, 1))
    in_maps = []
    for c in range(C):
        xT = np.zeros((P, npcp), dtype=np.float32)
        pos = sched[c]["pos_of_node"]
        xT[:, pos] = x[c * npc : (c + 1) * npc].T
        in_maps.append(
            {
                "xT": xT,
                "W1in": np.asarray(W1, np.float32),
                "W2in": np.asarray(W2, np.float32),
                "b1in": b1r,
                "b2in": b2r,
                "srcIdx": sched[c]["src_idx"],
                "dstRel": sched[c]["dst_rel"],
                "degNM": sched[c]["deg_nm"],
            }
        )
    return in_maps


def build_nc(cfg: Cfg2):
    C, TB, DH, DOUT = cfg.C, cfg.TB, cfg.dh, cfg.dout
    NG, NT, NB, npc, npcp = cfg.NG, cfg.NT, cfg.NB, cfg.npc, cfg.npcp

    nc = bacc.Bacc("TRN2", target_bir_lowering=False, debug=False, num_devices=C)

    xT = nc.dram_tensor("xT", [P, npcp], F32, kind="ExternalInput").ap()
    W1in = nc.dram_tensor("W1in", [P, DH], F32, kind="ExternalInput").ap()
    W2in = nc.dram_tensor("W2in", [DH, DOUT], F32, kind="ExternalInput").ap()
    b1in = nc.dram_tensor("b1in", [P, DH], F32, kind="ExternalInput").ap()
    b2in = nc.dram_tensor("b2in", [P, DOUT], F32, kind="ExternalInput").ap()
    srcIdx = nc.dram_tensor("srcIdx", [P, NB], I32, kind="ExternalInput").ap()
    dstRel = nc.dram_tensor("dstRel", [P, NB], F32, kind="ExternalInput").ap()
    degNM = nc.dram_tensor("degNM", [P, NT], F32, kind="ExternalInput").ap()
    z = nc.dram_tensor("z", [npcp, DOUT], F32, kind="ExternalOutput").ap()

    with tile.TileContext(nc) as tc:
        with (
            tc.tile_pool(name="const", bufs=1) as cpool,
            tc.tile_pool(name="work", bufs=1) as wpool,
            tc.tile_pool(name="psum", bufs=1, space="PSUM") as ppool,
            tc.tile_pool(name="dram", bufs=1, space="DRAM") as dpool,
        ):
            W1sb = cpool.tile([P, DH], F32)
            nc.sync.dma_start(W1sb[:], W1in[:])
            W2sb = cpool.tile([DH, DOUT], F32)
            nc.sync.dma_start(W2sb[:], W2in[:])
            b1sb = cpool.tile([P, DH], F32)
            nc.sync.dma_start(b1sb[:], b1in[:])
            b2sb = cpool.tile([P, DOUT], F32)
            nc.sync.dma_start(b2sb[:], b2in[:])
            ident = cpool.tile([P, P], F32)
            make_identity(nc, ident[:])
            iota_i = cpool.tile([P, WG], I32)
            nc.gpsimd.iota(iota_i[:], pattern=[[1, WG]], base=0, channel_multiplier=0)
            iota_f = cpool.tile([P, WG], F32)
            nc.vector.tensor_copy(iota_f[:], iota_i[:])
            src_sb = cpool.tile([P, NB], I32)
            nc.sync.dma_start(src_sb[:], srcIdx[:])
            dst_sb = cpool.tile([P, NB], F32)
            nc.sync.dma_start(dst_sb[:], dstRel[:])
            dinv = cpool.tile([P, NT], F32)
            nc.sync.dma_start(dinv[:], degNM[:])
            nc.scalar.activation(dinv[:], dinv[:], mybir.ActivationFunctionType.Sqrt)
            nc.vector.reciprocal(dinv[:], dinv[:])
            h1T = cpool.tile([P, NT * P], F32)  # [64 used, NT*128]

            G1s = dpool.tile([npcp, DH], F32)
            G1f = dpool.tile([C * npcp, DH], F32)
            G2s = dpool.tile([npcp, DOUT], F32)
            G2f = dpool.tile([C * npcp, DOUT], F32)
            rg = [list(range(C))]

            # layer 1 dense: G1 = dinv * (x @ W1), node-tile at a time
            for t in range(NT):
                ps = ppool.tile([P, DH], F32, tag="pdense", bufs=2, space="PSUM")
                xt_t = wpool.tile([P, P], F32, tag="xt", bufs=3)
                nc.sync.dma_start(xt_t[:], xT[:, t * P : (t + 1) * P])
                nc.tensor.matmul(ps[:], lhsT=xt_t[:], rhs=W1sb[:], start=True, stop=True)
                stage = wpool.tile([P, DH], F32, tag="g1stage", bufs=3)
                nc.vector.tensor_scalar_mul(stage[:], ps[:], dinv[:, t : t + 1])
                nc.sync.dma_start(G1s[t * P : (t + 1) * P, :], stage[:])

            nc.gpsimd.collective_compute(
                "AllGather", mybir.AluOpType.bypass, replica_groups=rg,
                ins=[G1s[:]], outs=[G1f[:]],
            )

            # layer 1 aggregation per 256-dst group
            for g in range(NG):
                rows = wpool.tile([P, TB * DH], F32, tag="rows1", bufs=2)
                for blk in range(TB):
                    col = g * TB + blk
                    nc.gpsimd.indirect_dma_start(
                        out=rows[:, blk * DH : (blk + 1) * DH],
                        out_offset=None,
                        in_=G1f[:],
                        in_offset=bass.IndirectOffsetOnAxis(
                            ap=src_sb[:, col : col + 1], axis=0
                        ),
                    )
                ps_lo = ppool.tile([P, DH], F32, tag="plo", bufs=2, space="PSUM")
                ps_hi = ppool.tile([P, DH], F32, tag="phi", bufs=2, space="PSUM")
                for blk in range(TB):
                    col = g * TB + blk
                    S = wpool.tile([P, WG], F32, tag="S", bufs=4)
                    nc.vector.tensor_tensor(
                        out=S[:],
                        in0=dst_sb[:, col : col + 1].to_broadcast([P, WG]),
                        in1=iota_f[:],
                        op=mybir.AluOpType.is_equal,
                    )
                    nc.tensor.matmul(
                        ps_lo[:], lhsT=S[:, 0:P],
                        rhs=rows[:, blk * DH : (blk + 1) * DH],
                        start=(blk == 0), stop=(blk == TB - 1),
                    )
                    nc.tensor.matmul(
                        ps_hi[:], lhsT=S[:, P:WG],
                        rhs=rows[:, blk * DH : (blk + 1) * DH],
                        start=(blk == 0), stop=(blk == TB - 1),
                    )
                for half, ps in ((0, ps_lo), (1, ps_hi)):
                    t = 2 * g + half
                    h1t = wpool.tile([P, DH], F32, tag="h1t", bufs=2)
                    nc.vector.tensor_scalar_mul(h1t[:], ps[:], dinv[:, t : t + 1])
                    nc.vector.tensor_add(h1t[:], h1t[:], b1sb[:])
                    nc.scalar.activation(
                        h1t[:], h1t[:], mybir.ActivationFunctionType.Relu
                    )
                    pt = ppool.tile([DH, P], F32, tag="ptr", bufs=2, space="PSUM")
                    nc.tensor.transpose(out=pt[:], in_=h1t[:], identity=ident[:])
                    nc.vector.tensor_copy(h1T[0:DH, t * P : (t + 1) * P], pt[:])

            # layer 2 dense: G2 = dinv * (h1 @ W2)
            for t in range(NT):
                ps2 = ppool.tile([P, DH], F32, tag="pdense", bufs=2, space="PSUM", name="ps2")
                nc.tensor.matmul(
                    ps2[:, :DOUT], lhsT=h1T[0:DH, t * P : (t + 1) * P],
                    rhs=W2sb[:], start=True, stop=True,
                )
                stage2 = wpool.tile([P, DOUT], F32, tag="g2stage", bufs=3)
                nc.vector.tensor_scalar_mul(stage2[:], ps2[:, :DOUT], dinv[:, t : t + 1])
                nc.sync.dma_start(G2s[t * P : (t + 1) * P, :], stage2[:])

            nc.gpsimd.collective_compute(
                "AllGather", mybir.AluOpType.bypass, replica_groups=rg,
                ins=[G2s[:]], outs=[G2f[:]],
            )

            # layer 2 aggregation + output
            for g in range(NG):
                rows2 = wpool.tile([P, TB * DOUT], F32, tag="rows2", bufs=2)
                for blk in range(TB):
                    col = g * TB + blk
                    nc.gpsimd.indirect_dma_start(
                        out=rows2[:, blk * DOUT : (blk + 1) * DOUT],
                        out_offset=None,
                        in_=G2f[:],
                        in_offset=bass.IndirectOffsetOnAxis(
                            ap=src_sb[:, col : col + 1], axis=0
                        ),
                    )
                ps_lo = ppool.tile([P, DH], F32, tag="plo", bufs=2, space="PSUM", name="ps_lo2")
                ps_hi = ppool.tile([P, DH], F32, tag="phi", bufs=2, space="PSUM", name="ps_hi2")
                for blk in range(TB):
                    col = g * TB + blk
                    S = wpool.tile([P, WG], F32, tag="S", bufs=4)
                    nc.vector.tensor_tensor(
                        out=S[:],
                        in0=dst_sb[:, col : col + 1].to_broadcast([P, WG]),
                        in1=iota_f[:],
                        op=mybir.AluOpType.is_equal,
                    )
                    nc.tensor.matmul(
                        ps_lo[:, :DOUT], lhsT=S[:, 0:P],
                        rhs=rows2[:, blk * DOUT : (blk + 1) * DOUT],
                        start=(blk == 0), stop=(blk == TB - 1),
                    )
                    nc.tensor.matmul(
                        ps_hi[:, :DOUT], lhsT=S[:, P:WG],
                        rhs=rows2[:, blk * DOUT : (blk + 1) * DOUT],
                        start=(blk == 0), stop=(blk == TB - 1),
                    )
                for half, ps in ((0, ps_lo), (1, ps_hi)):
                    t = 2 * g + half
                    zs = wpool.tile([P, DOUT], F32, tag="zs", bufs=2)
                    nc.vector.tensor_scalar_mul(zs[:], ps[:, :DOUT], dinv[:, t : t + 1])
                    nc.vector.tensor_add(zs[:], zs[:], b2sb[:])
                    nc.sync.dma_start(z[t * P : (t + 1) * P, :], zs[:])
    nc.compile()
    return nc


def reference_np(x, W1, b1, W2, b2, edge_index):
    n = x.shape[0]
    src = np.concatenate([edge_index[0], np.arange(n)])
    dst = np.concatenate([edge_index[1], np.arange(n)])
    deg = np.bincount(dst, minlength=n).astype(np.float32)
    dinv = np.where(deg > 0, 1.0 / np.sqrt(deg), 0.0)
    norm = dinv[src] * dinv[dst]

    def conv(h, Wm, bv):
        h = h @ Wm
        msg = h[src] * norm[:, None]
        agg = np.zeros((n, h.shape[1]), np.float32)
        np.add.at(agg, dst, msg)
        return agg + bv

    h = np.maximum(conv(x, W1, b1), 0.0)
    return conv(h, W2, b2)


N_CORES = 8
_NC_CACHE = {}


def _cached_nc(cfg):
    if cfg not in _NC_CACHE:
        _NC_CACHE[cfg] = build_nc(cfg)
    return _NC_CACHE[cfg]


def kernel(x, W1, b1, W2, b2, edge_index):
    x = np.asarray(x)
    n = x.shape[0]
    cfg, sched = build_schedule(np.asarray(edge_index), n, N_CORES)
    in_maps = build_inputs(cfg, x, W1, b1, W2, b2, sched)
    nc = _cached_nc(cfg)
    res = bass_utils.run_bass_kernel_spmd(nc, in_maps, core_ids=list(range(N_CORES)))
    z = np.concatenate(
        [res.results[c]["z"][sched[c]["pos_of_node"]] for c in range(N_CORES)], axis=0
    )
    return z.astype(np.float32)



# revision 2
# speedup vs baseline: 808.3921x; 808.3921x over previous
"""Distributed 2-layer GCN (GCLEncoder) on 8 Trainium2 NeuronCores — Bass/Tile.

kernel(**inputs) takes the FULL inputs (x [100000,128] f32, W1 [128,64],
b1 [64], W2 [64,32], b2 [32], edge_index [2,1600000] i32) and returns the
FULL output z [100000, 32] f32.

Destination nodes are sharded contiguously across the 8 cores; per-layer
feature tables G = dinv*(H@W) are node-sharded and exchanged via AllGather.
Within a core, nodes are assigned to 256-destination groups by a
degree-balancing permutation so every group needs the same minimal number
of 128-edge blocks (TB = ceil(mean/128)); the host un-permutes the output
rows.  Per block: indirect-DMA gather of referenced table rows (128
rows/instruction — the only gather primitive on this runtime), a 256-wide
one-hot S on the vector engine, and two M=128 matmuls (lo/hi halves)
accumulating in PSUM.  Self-loops are explicit edges; degrees are
host-computed index metadata; all float math runs on device.
"""


from dataclasses import dataclass

import numpy as np

import concourse.bass as bass
import concourse.tile as tile
import concourse.bacc as bacc
from concourse import bass_utils, mybir
from concourse.masks import make_identity

F32 = mybir.dt.float32
I32 = mybir.dt.int32
P = 128
WG = 256  # dst group size


@dataclass(frozen=True)
class Cfg2:
    n_nodes: int
    din: int
    dh: int
    dout: int
    C: int
    NG: int  # 256-dst groups per core
    TB: int  # 128-slot blocks per group

    @property
    def npc(self):
        return self.n_nodes // self.C

    @property
    def npcp(self):
        return self.NG * WG

    @property
    def NT(self):  # 128-node tiles per core
        return self.NG * 2

    @property
    def NB(self):
        return self.NG * self.TB


def build_schedule(edge_index: np.ndarray, n_nodes: int, C: int):
    src = np.asarray(edge_index[0], dtype=np.int64)
    dst = np.asarray(edge_index[1], dtype=np.int64)
    loops = np.arange(n_nodes, dtype=np.int64)
    src = np.concatenate([src, loops])
    dst = np.concatenate([dst, loops])

    npc = n_nodes // C
    NG = -(-npc // WG)
    npcp = NG * WG

    deg = np.bincount(dst, minlength=n_nodes).astype(np.float32)
    owner = dst // npc

    # balance nodes into groups by degree so the per-group padded block
    # count TB = ceil(max group count / 128) sits at the mean, not the max
    import heapq

    pos_of_node = np.zeros((C, npc), dtype=np.int64)
    node_of_pos = np.full((C, npcp), -1, dtype=np.int64)
    for c in range(C):
        dc = deg[c * npc : (c + 1) * npc].astype(np.int64)
        order = np.argsort(-dc, kind="stable")
        sums = [0.0] * NG
        cnts = [0] * NG
        heap = [(0.0, b) for b in range(NG)]
        heapq.heapify(heap)
        binof = np.zeros(npc, dtype=np.int64)
        for nidx in order:
            while True:
                sm, b = heapq.heappop(heap)
                if cnts[b] < WG:
                    break
            binof[nidx] = b
            cnts[b] += 1
            sums[b] += dc[nidx]
            if cnts[b] < WG:
                heapq.heappush(heap, (sums[b], b))
        # refinement: swap between max and min bins to shave the max
        for _ in range(4000):
            bmax = int(np.argmax(sums))
            bmin = int(np.argmin(sums))
            gap = sums[bmax] - sums[bmin]
            if gap <= 1:
                break
            in_max = np.where(binof == bmax)[0]
            in_min = np.where(binof == bmin)[0]
            du = dc[in_max]
            dv = dc[in_min]
            diff = du[:, None] - dv[None, :]
            good = (diff > 0) & (diff < gap)
            if not good.any():
                break
            tgt = gap / 2.0
            score = np.where(good, np.abs(diff - tgt), np.inf)
            iu, iv = np.unravel_index(np.argmin(score), score.shape)
            u, v = in_max[iu], in_min[iv]
            d_ = int(dc[u] - dc[v])
            binof[u], binof[v] = bmin, bmax
            sums[bmax] -= d_
            sums[bmin] += d_
        # positions: pack each bin's nodes then ghosts
        slot = np.zeros(NG, dtype=np.int64)
        for nidx in range(npc):
            b = binof[nidx]
            p = b * WG + slot[b]
            slot[b] += 1
            pos_of_node[c, nidx] = p
            node_of_pos[c, p] = nidx

    per_core = []
    maxcnt = 0
    for c in range(C):
        m = owner == c
        s_glob = src[m]
        d = dst[m] - c * npc
        pd = pos_of_node[c, d]
        order = np.argsort(pd, kind="stable")
        s_glob, pd = s_glob[order], pd[order]
        t_idx = (s_glob // npc) * npcp + pos_of_node[s_glob // npc, s_glob % npc]
        grp = pd // WG
        counts = np.bincount(grp, minlength=NG)
        maxcnt = max(maxcnt, int(counts.max()))
        per_core.append((t_idx, pd, grp, counts))

    TB = max(1, -(-maxcnt // P))
    cfg = Cfg2(n_nodes=n_nodes, din=128, dh=64, dout=32, C=C, NG=NG, TB=TB)

    data = []
    for c in range(C):
        t_idx, pd, grp, counts = per_core[c]
        NB = NG * TB
        src_idx = np.zeros((P, NB), dtype=np.int32)
        dst_rel = np.full((P, NB), -1.0, dtype=np.float32)
        starts = np.cumsum(counts) - counts
        pos = np.arange(len(pd)) - starts[grp]
        col = grp * TB + pos // P
        row = pos % P
        src_idx[row, col] = t_idx.astype(np.int32)
        dst_rel[row, col] = (pd - grp * WG).astype(np.float32)

        deg_nm = np.ones((P, NG * 2), dtype=np.float32)
        valid = node_of_pos[c] >= 0
        dpad = np.ones(npcp, np.float32)
        dpad[valid] = deg[c * npc + node_of_pos[c][valid]]
        deg_nm[:, :] = dpad.reshape(NG * 2, P).T
        data.append(
            {
                "src_idx": src_idx,
                "dst_rel": dst_rel,
                "deg_nm": deg_nm,
                "pos_of_node": pos_of_node[c],
            }
        )
    return cfg, data


def build_inputs(cfg: Cfg2, x, W1, b1, W2, b2, sched):
    C, npc, npcp = cfg.C, cfg.npc, cfg.npcp
    x = np.asarray(x, dtype=np.float32)
    b1r = np.tile(np.asarray(b1, np.float32)[None, :], (P, 1))
    b2r = np.tile(np.asarray(b2, np.float32)[None, :], (P, 1))
    in_maps = []
    for c in range(C):
        xT = np.zeros((P, npcp), dtype=np.float32)
        pos = sched[c]["pos_of_node"]
        xT[:, pos] = x[c * npc : (c + 1) * npc].T
        in_maps.append(
            {
                "xT": xT,
                "W1in": np.asarray(W1, np.float32),
                "W2in": np.asarray(W2, np.float32),
                "b1in": b1r,
                "b2in": b2r,
                "srcIdx": sched[c]["src_idx"],
                "dstRel": sched[c]["dst_rel"],
                "degNM": sched[c]["deg_nm"],
            }
        )
    return in_maps


def build_nc(cfg: Cfg2):
    C, TB, DH, DOUT = cfg.C, cfg.TB, cfg.dh, cfg.dout
    NG, NT, NB, npc, npcp = cfg.NG, cfg.NT, cfg.NB, cfg.npc, cfg.npcp

    nc = bacc.Bacc("TRN2", target_bir_lowering=False, debug=False, num_devices=C)

    xT = nc.dram_tensor("xT", [P, npcp], F32, kind="ExternalInput").ap()
    W1in = nc.dram_tensor("W1in", [P, DH], F32, kind="ExternalInput").ap()
    W2in = nc.dram_tensor("W2in", [DH, DOUT], F32, kind="ExternalInput").ap()
    b1in = nc.dram_tensor("b1in", [P, DH], F32, kind="ExternalInput").ap()
    b2in = nc.dram_tensor("b2in", [P, DOUT], F32, kind="ExternalInput").ap()
    srcIdx = nc.dram_tensor("srcIdx", [P, NB], I32, kind="ExternalInput").ap()
    dstRel = nc.dram_tensor("dstRel", [P, NB], F32, kind="ExternalInput").ap()
    degNM = nc.dram_tensor("degNM", [P, NT], F32, kind="ExternalInput").ap()
    z = nc.dram_tensor("z", [npcp, DOUT], F32, kind="ExternalOutput").ap()

    with tile.TileContext(nc) as tc:
        with (
            tc.tile_pool(name="const", bufs=1) as cpool,
            tc.tile_pool(name="work", bufs=1) as wpool,
            tc.tile_pool(name="psum", bufs=1, space="PSUM") as ppool,
            tc.tile_pool(name="dram", bufs=1, space="DRAM") as dpool,
        ):
            W1sb = cpool.tile([P, DH], F32)
            nc.sync.dma_start(W1sb[:], W1in[:])
            W2sb = cpool.tile([DH, DOUT], F32)
            nc.sync.dma_start(W2sb[:], W2in[:])
            b1sb = cpool.tile([P, DH], F32)
            nc.sync.dma_start(b1sb[:], b1in[:])
            b2sb = cpool.tile([P, DOUT], F32)
            nc.sync.dma_start(b2sb[:], b2in[:])
            ident = cpool.tile([P, P], F32)
            make_identity(nc, ident[:])
            iota_i = cpool.tile([P, WG], I32)
            nc.gpsimd.iota(iota_i[:], pattern=[[1, WG]], base=0, channel_multiplier=0)
            iota_f = cpool.tile([P, WG], F32)
            nc.vector.tensor_copy(iota_f[:], iota_i[:])
            src_sb = cpool.tile([P, NB], I32)
            nc.sync.dma_start(src_sb[:], srcIdx[:])
            dst_sb = cpool.tile([P, NB], F32)
            nc.sync.dma_start(dst_sb[:], dstRel[:])
            dinv = cpool.tile([P, NT], F32)
            nc.sync.dma_start(dinv[:], degNM[:])
            nc.scalar.activation(dinv[:], dinv[:], mybir.ActivationFunctionType.Sqrt)
            nc.vector.reciprocal(dinv[:], dinv[:])
            h1T = cpool.tile([P, NT * P], F32)  # [64 used, NT*128]

            G1s = dpool.tile([npcp, DH], F32)
            G1f = dpool.tile([C * npcp, DH], F32)
            G2s = dpool.tile([npcp, DOUT], F32)
            G2f = dpool.tile([C * npcp, DOUT], F32)
            rg = [list(range(C))]

            # layer 1 dense: G1 = dinv * (x @ W1), node-tile at a time
            for t in range(NT):
                ps = ppool.tile([P, DH], F32, tag="pdense", bufs=2, space="PSUM")
                xt_t = wpool.tile([P, P], F32, tag="xt", bufs=3)
                nc.sync.dma_start(xt_t[:], xT[:, t * P : (t + 1) * P])
                nc.tensor.matmul(ps[:], lhsT=xt_t[:], rhs=W1sb[:], start=True, stop=True)
                stage = wpool.tile([P, DH], F32, tag="g1stage", bufs=3)
                nc.vector.tensor_scalar_mul(stage[:], ps[:], dinv[:, t : t + 1])
                nc.sync.dma_start(G1s[t * P : (t + 1) * P, :], stage[:])

            nc.gpsimd.collective_compute(
                "AllGather", mybir.AluOpType.bypass, replica_groups=rg,
                ins=[G1s[:]], outs=[G1f[:]],
            )

            # layer 1 aggregation per 256-dst group
            for g in range(NG):
                rows = wpool.tile([P, TB * DH], F32, tag="rows1", bufs=2)
                for blk in range(TB):
                    col = g * TB + blk
                    nc.gpsimd.indirect_dma_start(
                        out=rows[:, blk * DH : (blk + 1) * DH],
                        out_offset=None,
                        in_=G1f[:],
                        in_offset=bass.IndirectOffsetOnAxis(
                            ap=src_sb[:, col : col + 1], axis=0
                        ),
                    )
                ps_lo = ppool.tile([P, DH], F32, tag="plo", bufs=2, space="PSUM")
                ps_hi = ppool.tile([P, DH], F32, tag="phi", bufs=2, space="PSUM")
                for blk in range(TB):
                    col = g * TB + blk
                    S = wpool.tile([P, WG], F32, tag="S", bufs=4)
                    nc.vector.tensor_tensor(
                        out=S[:],
                        in0=dst_sb[:, col : col + 1].to_broadcast([P, WG]),
                        in1=iota_f[:],
                        op=mybir.AluOpType.is_equal,
                    )
                    nc.tensor.matmul(
                        ps_lo[:], lhsT=S[:, 0:P],
                        rhs=rows[:, blk * DH : (blk + 1) * DH],
                        start=(blk == 0), stop=(blk == TB - 1),
                    )
                    nc.tensor.matmul(
                        ps_hi[:], lhsT=S[:, P:WG],
                        rhs=rows[:, blk * DH : (blk + 1) * DH],
                        start=(blk == 0), stop=(blk == TB - 1),
                    )
                for half, ps in ((0, ps_lo), (1, ps_hi)):
                    t = 2 * g + half
                    h1t = wpool.tile([P, DH], F32, tag="h1t", bufs=2)
                    nc.vector.tensor_scalar_mul(h1t[:], ps[:], dinv[:, t : t + 1])
                    nc.vector.tensor_add(h1t[:], h1t[:], b1sb[:])
                    nc.scalar.activation(
                        h1t[:], h1t[:], mybir.ActivationFunctionType.Relu
                    )
                    pt = ppool.tile([DH, P], F32, tag="ptr", bufs=2, space="PSUM")
                    nc.tensor.transpose(out=pt[:], in_=h1t[:], identity=ident[:])
                    nc.vector.tensor_copy(h1T[0:DH, t * P : (t + 1) * P], pt[:])

            # layer 2 dense: G2 = dinv * (h1 @ W2)
            for t in range(NT):
                ps2 = ppool.tile([P, DH], F32, tag="pdense", bufs=2, space="PSUM", name="ps2")
                nc.tensor.matmul(
                    ps2[:, :DOUT], lhsT=h1T[0:DH, t * P : (t + 1) * P],
                    rhs=W2sb[:], start=True, stop=True,
                )
                stage2 = wpool.tile([P, DOUT], F32, tag="g2stage", bufs=3)
                nc.vector.tensor_scalar_mul(stage2[:], ps2[:, :DOUT], dinv[:, t : t + 1])
                nc.sync.dma_start(G2s[t * P : (t + 1) * P, :], stage2[:])

            nc.gpsimd.collective_compute(
                "AllGather", mybir.AluOpType.bypass, replica_groups=rg,
                ins=[G2s[:]], outs=[G2f[:]],
            )

            # layer 2 aggregation + output
            for g in range(NG):
                rows2 = wpool.tile([P, TB * DOUT], F32, tag="rows2", bufs=2)
                for blk in range(TB):
                    col = g * TB + blk
                    nc.gpsimd.indirect_dma_start(
                        out=rows2[:, blk * DOUT : (blk + 1) * DOUT],
                        out_offset=None,
                        in_=G2f[:],
                        in_offset=bass.IndirectOffsetOnAxis(
                            ap=src_sb[:, col : col + 1], axis=0
                        ),
                    )
                ps_lo = ppool.tile([P, DH], F32, tag="plo", bufs=2, space="PSUM", name="ps_lo2")
                ps_hi = ppool.tile([P, DH], F32, tag="phi", bufs=2, space="PSUM", name="ps_hi2")
                for blk in range(TB):
                    col = g * TB + blk
                    S = wpool.tile([P, WG], F32, tag="S", bufs=4)
                    nc.vector.tensor_tensor(
                        out=S[:],
                        in0=dst_sb[:, col : col + 1].to_broadcast([P, WG]),
                        in1=iota_f[:],
                        op=mybir.AluOpType.is_equal,
                    )
                    nc.tensor.matmul(
                        ps_lo[:, :DOUT], lhsT=S[:, 0:P],
                        rhs=rows2[:, blk * DOUT : (blk + 1) * DOUT],
                        start=(blk == 0), stop=(blk == TB - 1),
                    )
                    nc.tensor.matmul(
                        ps_hi[:, :DOUT], lhsT=S[:, P:WG],
                        rhs=rows2[:, blk * DOUT : (blk + 1) * DOUT],
                        start=(blk == 0), stop=(blk == TB - 1),
                    )
                for half, ps in ((0, ps_lo), (1, ps_hi)):
                    t = 2 * g + half
                    zs = wpool.tile([P, DOUT], F32, tag="zs", bufs=2)
                    nc.vector.tensor_scalar_mul(zs[:], ps[:, :DOUT], dinv[:, t : t + 1])
                    nc.vector.tensor_add(zs[:], zs[:], b2sb[:])
                    nc.sync.dma_start(z[t * P : (t + 1) * P, :], zs[:])
    nc.compile()
    return nc


def reference_np(x, W1, b1, W2, b2, edge_index):
    n = x.shape[0]
    src = np.concatenate([edge_index[0], np.arange(n)])
    dst = np.concatenate([edge_index[1], np.arange(n)])
    deg = np.bincount(dst, minlength=n).astype(np.float32)
    dinv = np.where(deg > 0, 1.0 / np.sqrt(deg), 0.0)
    norm = dinv[src] * dinv[dst]

    def conv(h, Wm, bv):
        h = h @ Wm
        msg = h[src] * norm[:, None]
        agg = np.zeros((n, h.shape[1]), np.float32)
        np.add.at(agg, dst, msg)
        return agg + bv

    h = np.maximum(conv(x, W1, b1), 0.0)
    return conv(h, W2, b2)


N_CORES = 8
_NC_CACHE = {}


def _cached_nc(cfg):
    if cfg not in _NC_CACHE:
        _NC_CACHE[cfg] = build_nc(cfg)
    return _NC_CACHE[cfg]


def assemble_output(cfg, sched, results):
    return np.concatenate(
        [results[c]["z"][sched[c]["pos_of_node"]] for c in range(cfg.C)], axis=0
    ).astype(np.float32)


def kernel(x, W1, b1, W2, b2, edge_index):
    x = np.asarray(x)
    n = x.shape[0]
    cfg, sched = build_schedule(np.asarray(edge_index), n, N_CORES)
    in_maps = build_inputs(cfg, x, W1, b1, W2, b2, sched)
    nc = _cached_nc(cfg)
    res = bass_utils.run_bass_kernel_spmd(nc, in_maps, core_ids=list(range(N_CORES)))
    return assemble_output(cfg, sched, res.results)

